# revision 1
# baseline (speedup 1.0000x reference)
"""Multi-head attention (B=2, S=2048, D=1024, H=16, causal) on 8 Trainium2 cores.

Sharding: core c handles batch b = c // 4 and head group g = c % 4 (4 heads,
d_model column slice [256*g, 256*g+256)).  QKV projections are computed per
core against the full sequence of its batch; attention runs per head in a
"scores-transposed" [k, q] layout which makes every matmul operand land in
its natural layout (no on-device transposes of activations beyond the initial
DMA-xbar transpose of x); the output projection produces a per-core partial
[S, D] that the host sums over the 4 head-group cores of each batch.

All matmul operands are bf16 (host pre-casts); accumulation is fp32 in PSUM,
softmax runs in fp32 on the ACT engine (exp with the 1/sqrt(dh) scale fused).
The softmax denominator comes for free from 64 ones-columns appended to V:
P@V output rows 64:127 all hold the denominator, so the reciprocal is already
broadcast across partitions for the normalize multiply.

Perf structure (v2):
- x transposes split into [1024, 128] pieces alternated across both HWDGE
  rings (nc.sync / nc.scalar) so the first projection unblocks in ~6us.
- Weights/biases/strips load via SWDGE (nc.gpsimd) to keep HWDGE rings free.
- Attention is slab-outer/head-inner with the output projection emitted per
  slab, spreading output stores across the run.
- Scores matmuls for a head pair are d-interleaved: heads 2h/2h+1 occupy PE
  row groups 0-63/64-127, so consecutive K=64 matmuls run concurrently.
"""

import functools
import numpy as np
import ml_dtypes

import concourse.bass as bass
import concourse.bacc as bacc
import concourse.tile as tile
import concourse.mybir as mybir
from concourse.bass_utils import run_bass_kernel_spmd

dt = mybir.dt
F32 = dt.float32
BF16 = dt.bfloat16
FP8 = dt.float8e4
AFT = mybir.ActivationFunctionType

B, S, D = 2, 2048, 1024
H, DH = 16, 64
NCORES = 8
GROUPS = NCORES // B            # 4 head-groups
HC = H // GROUPS                # 4 heads per core
C = HC * DH                     # 256 = per-core head-column slice
P = 128
DK = D // P                     # 8 d_in chunks
SB = 512                        # q-slab width
NSLAB = S // SB                 # 4
KT = S // P                     # 16 k tiles
SHALF = 2                       # transpose piece split along s
SCALE = 1.0 / float(np.sqrt(DH))


def _build(mask_mode: str, dump: bool = False):
    """mask_mode: 'causal' | 'none' | 'generic'. Returns compiled Bacc."""
    assert mask_mode in ("causal", "none", "generic")
    nc = bacc.Bacc("TRN2", target_bir_lowering=False, debug=False)

    XS = S // SHALF
    xq_d = nc.dram_tensor("xq", [DK, SHALF, XS, P], BF16, kind="ExternalInput").ap()
    xk_d = nc.dram_tensor("xk", [DK, SHALF, XS, P], BF16, kind="ExternalInput").ap()
    xv_d = nc.dram_tensor("xv", [DK, SHALF, XS, P], BF16, kind="ExternalInput").ap()
    # all bf16 constants packed into one tensor (one DMA): wq|wk|wv|wo|strips
    cb_d = nc.dram_tensor("cb", [P, 5 * 2048], BF16, kind="ExternalInput").ap()
    cf_d = nc.dram_tensor("cf", [P, 260], F32, kind="ExternalInput").ap()
    if mask_mode == "generic":
        maskT_d = nc.dram_tensor("maskT", [S, S], BF16, kind="ExternalInput").ap()
    o_d = nc.dram_tensor("o", [S, D], F32, kind="ExternalOutput").ap()
    if dump:
        xqT_o = nc.dram_tensor("xqT_o", [P, DK, S], BF16, kind="ExternalOutput").ap()
        qT_o = nc.dram_tensor("qT_o", [P, C // P, S], BF16, kind="ExternalOutput").ap()
        kT_o = nc.dram_tensor("kT_o", [P, C // P, S], BF16, kind="ExternalOutput").ap()
        v_o = nc.dram_tensor("v_o", [P, KT, HC, P], BF16, kind="ExternalOutput").ap()
        headsT_o = nc.dram_tensor("headsT_o", [P, C // P, S], BF16,
                                  kind="ExternalOutput").ap()

    with tile.TileContext(nc) as tc:
        with (
            tc.tile_pool(name="consts", bufs=1) as consts,
            tc.tile_pool(name="xT", bufs=3) as xT_pool,
            tc.tile_pool(name="acts", bufs=1) as acts,
            tc.tile_pool(name="expT", bufs=8) as exp_pool,
            tc.tile_pool(name="stage", bufs=2) as stage,
            tc.tile_pool(name="pp", bufs=2, space="PSUM") as pp,
            tc.tile_pool(name="sps", bufs=2, space="PSUM") as sps,
            tc.tile_pool(name="otp", bufs=2, space="PSUM") as otp,
        ):
            # ---- packed constants: two DMAs up front on the sync ring
            cb_sb = consts.tile([P, 5 * 2048], BF16)
            cf_sb = consts.tile([P, 260], F32)
            nc.gpsimd.dma_start(cb_sb[:], cb_d)
            nc.gpsimd.dma_start(cf_sb[:], cf_d)
            wq_sb = cb_sb[:, 0:2048].rearrange("p (o n) -> p o n", o=DK)
            wk_sb = cb_sb[:, 2048:4096].rearrange("p (o n) -> p o n", o=DK)
            wv_sb = cb_sb[:, 4096:6144].rearrange("p (o n) -> p o n", o=DK)
            wo_sb = cb_sb[:, 6144:8192].rearrange("p (c n) -> p c n", c=C // P)
            strips_sb = cb_sb[:, 8192:10240].rearrange("p (i n) -> p i n",
                                                       i=SB // P)
            bq_sb = cf_sb[:, 0:2]
            bk_sb = cf_sb[:, 2:4]
            bvb_sb = cf_sb[:, 4:260]
            expbias_sb = consts.tile([P, 1], F32)
            nc.vector.memset(expbias_sb[:], -3.0)

            # ---- x transposes: all on the sync HWDGE ring (concurrent xbar
            # transposes on the two rings corrupt each other).  sh-major and
            # tensor-interleaved so the s-half-0 pieces of q, k AND v land
            # first and slab-0 attention unblocks early.
            xqT = xT_pool.tile([P, DK, S], BF16, tag="xT", name="xqT")
            xkT = xT_pool.tile([P, DK, S], BF16, tag="xT", name="xkT")
            xvT = xT_pool.tile([P, DK, S], BF16, tag="xT", name="xvT")
            XS = S // SHALF
            for sh in range(SHALF):
                for (x_d, xt) in ((xq_d, xqT), (xk_d, xkT), (xv_d, xvT)):
                    for o in range(DK):
                        nc.sync.dma_start_transpose(
                            xt[:, o, sh * XS:(sh + 1) * XS], x_d[o, sh])

            qT_sb = acts.tile([P, C // P, S], BF16)       # [d_out, s] head-major
            kT_sb = acts.tile([P, C // P, S], BF16)
            headsT_sb = acts.tile([P, C // P, S], BF16)
            # v pair-indexed: [p, kpair, i, h, col]; cols 0:64 v, 64:128 ones
            v_sb = acts.tile([P, KT // 2, 2, HC, P], BF16)
            nc.vector.memset(v_sb[:, :, :, :, DH:P], 1.0)

            def proj_qk(jpair):
                # j-paired emission: both matmuls of a j-pair share the same
                # stationary weight chunk (one LDWEIGHTS each pair).
                for (w_sb, b_sb, outT, xT) in ((wq_sb, bq_sb, qT_sb, xqT),
                                               (wk_sb, bk_sb, kT_sb, xkT)):
                    for co in range(C // P):
                        ps = {}
                        for j in jpair:
                            ps[j] = pp.tile([P, SB], F32, tag="pp",
                                            name="proj_ps")
                        for o in range(DK):
                            for j in jpair:
                                nc.tensor.matmul(
                                    ps[j][:],
                                    lhsT=w_sb[:, o, co * P:(co + 1) * P],
                                    rhs=xT[:, o, j * SB:(j + 1) * SB],
                                    start=(o == 0), stop=(o == DK - 1))
                        for j in jpair:
                            nc.vector.tensor_scalar_add(
                                outT[:, co, j * SB:(j + 1) * SB], ps[j][:],
                                b_sb[:, co:co + 1])

            def vproj(st_range):
                for st in st_range:
                    ps = pp.tile([P, SB], F32, tag="pp", name="vproj_ps")
                    for o in range(DK):
                        nc.tensor.matmul(
                            ps[:, 0:C],
                            lhsT=xvT[:, o, st * P:(st + 1) * P],
                            rhs=wv_sb[:, o, :],
                            start=(o == 0), stop=(o == DK - 1))
                    nc.vector.tensor_add(
                        v_sb[:, st // 2, st % 2, :, 0:DH],
                        ps[:, 0:C].rearrange("p (h d) -> p h d", h=HC),
                        bvb_sb[:].rearrange("p (h d) -> p h d", h=HC))

            def attn_slab(j):
                n_kt = 4 * (j + 1) if mask_mode == "causal" else KT
                for hc in range(HC // 2):          # head pair (2hc, 2hc+1)
                    outp = [otp.tile([P, SB], F32, tag="otp", name=f"outp{hh}")
                            for hh in range(2)]
                    for tb in range(0, n_kt, 2):
                        npair = min(2, n_kt - tb)

                        def qlo(t):   # valid-q offset within slab for k-tile t
                            if mask_mode != "causal":
                                return 0
                            return max(0, P * t - SB * j)

                        sp = [sps.tile([P, 2, SB], F32, tag="sps",
                                       name=f"sp{hh}") for hh in range(2)]
                        expT = [exp_pool.tile([P, 2, SB], BF16, tag="expT",
                                              name=f"expT{hh}")
                                for hh in range(2)]
                        for d_ in range(npair):
                            t = tb + d_
                            ql = qlo(t)
                            for hh in range(2):
                                hp = DH * hh
                                nc.tensor.matmul(
                                    sp[hh][:, d_, ql:],
                                    lhsT=kT_sb[hp:hp + DH, hc,
                                               t * P:(t + 1) * P],
                                    rhs=qT_sb[hp:hp + DH, hc,
                                              j * SB + ql:(j + 1) * SB],
                                    start=True, stop=True)
                        qb = qlo(tb)
                        for hh in range(2):
                            # bias -3: keeps exp within fp8e4 range (softmax
                            # is shift-invariant; numerator and denominator
                            # share the e^-3 factor)
                            nc.scalar.activation(
                                expT[hh][:, 0:npair, qb:],
                                sp[hh][:, 0:npair, qb:], AFT.Exp, scale=SCALE,
                                bias=expbias_sb[:])
                        assert npair == 2
                        qp = qlo(tb)      # pair-level valid-q offset
                        for d_ in range(npair):
                            t = tb + d_
                            if mask_mode == "generic":
                                m_sb = stage.tile([P, SB], BF16, tag="msk",
                                                  name="m_sb")
                                nc.gpsimd.dma_start(
                                    m_sb[:],
                                    maskT_d[t * P:(t + 1) * P,
                                            j * SB:(j + 1) * SB])
                            for hh in range(2):
                                if mask_mode == "causal" and t >= 4 * j:
                                    # mask relative to the PAIR's q window:
                                    # d_=0: triangle on 128 cols; d_=1: zero
                                    # prefix + triangle over 256 cols
                                    w = min((d_ + 1) * P, SB - qp)
                                    nc.vector.tensor_mul(
                                        expT[hh][:, d_, qp:qp + w],
                                        expT[hh][:, d_, qp:qp + w],
                                        strips_sb[:, d_, 0:w])
                                elif mask_mode == "generic":
                                    nc.vector.tensor_mul(
                                        expT[hh][:, d_, :], expT[hh][:, d_, :],
                                        m_sb[:])
                        for hh in range(2):
                            h = 2 * hc + hh
                            for d_ in range(npair):
                                t = tb + d_
                                ql = qlo(t)
                                nc.tensor.matmul(
                                    outp[hh][:, ql:],
                                    lhsT=v_sb[:, tb // 2, d_, h, :],
                                    rhs=expT[hh][:, d_, ql:],
                                    start=(t == 0), stop=(t == n_kt - 1))
                    # normalize: rows 64:128 of outp hold the denominator.
                    # reciprocal_approx_fast mis-reads when in/out partition
                    # bases differ, so run it over all 128 rows (rows 0:64
                    # are recip of the unnormalized output — unused).
                    for hh in range(2):
                        hp = DH * hh
                        recip = stage.tile([P, SB], F32, tag="recip",
                                           name="recip")
                        nc.vector.reciprocal_approx_fast(recip[:], outp[hh][:])
                        nc.vector.tensor_mul(
                            headsT_sb[hp:hp + DH, hc, j * SB:(j + 1) * SB],
                            outp[hh][0:DH, :], recip[DH:P, :])

            def oproj_slab(j):
                for st in range(4 * j, 4 * j + 4):
                    ob = stage.tile([P, D], F32, tag="ob", name="ob")
                    ps = {}
                    for n2 in range(D // SB):
                        ps[n2] = pp.tile([P, SB], F32, tag="pp", name="o_ps")
                    # cc-outer: both n2 matmuls of a cc share the stationary
                    for cc in range(C // P):
                        for n2 in range(D // SB):
                            nc.tensor.matmul(
                                ps[n2][:],
                                lhsT=headsT_sb[:, cc, st * P:(st + 1) * P],
                                rhs=wo_sb[:, cc, n2 * SB:(n2 + 1) * SB],
                                start=(cc == 0), stop=(cc == C // P - 1))
                    for n2 in range(D // SB):
                        if (st + n2) % 2 == 0:
                            nc.vector.tensor_copy(ob[:, n2 * SB:(n2 + 1) * SB],
                                                  ps[n2][:])
                        else:
                            nc.scalar.copy(ob[:, n2 * SB:(n2 + 1) * SB],
                                           ps[n2][:])
                    nc.scalar.dma_start(o_d[st * P:(st + 1) * P, :], ob[:])

            # ---- interleaved schedule: projections feed attention per slab
            proj_qk((0, 1))
            vproj(range(0, 8))
            attn_slab(0)
            oproj_slab(0)
            proj_qk((2, 3))
            attn_slab(1)
            oproj_slab(1)
            vproj(range(8, KT))
            attn_slab(2)
            oproj_slab(2)
            attn_slab(3)
            oproj_slab(3)

    nc.compile()
    return nc


@functools.lru_cache(maxsize=4)
def _get(mask_mode: str):
    return _build(mask_mode)


def _bf16(a):
    return np.ascontiguousarray(a.astype(ml_dtypes.bfloat16))


def _detect_mask_mode(m):
    if (m == 1).all():
        return "none"
    idx = np.arange(m.shape[0])
    if np.array_equal(m != 0, idx[None, :] <= idx[:, None]):
        return "causal"
    return "generic"


def _strips():
    p = np.arange(P)[:, None]
    f = np.arange(SB)[None, :]
    s = np.stack([(p <= f - P * i) for i in range(SB // P)], axis=1)
    return np.ascontiguousarray(s.astype(ml_dtypes.bfloat16))


def prepare(query, key, value, mask, Wq, bq, Wk, bk, Wv, bv, Wo, bo):
    """Returns (mask_mode, in_maps) for run_bass_kernel_spmd."""
    query = np.asarray(query, dtype=np.float32)
    key = np.asarray(key, dtype=np.float32)
    value = np.asarray(value, dtype=np.float32)
    m2d = np.asarray(mask).reshape(np.asarray(mask).shape[-2:])
    mask_mode = _detect_mask_mode(m2d)

    def prep_x(x):    # [S, D] -> contiguous pieces [DK, SHALF, S//SHALF, P]
        return _bf16(x.reshape(SHALF, S // SHALF, DK, P).transpose(2, 0, 1, 3))

    xq = [prep_x(query[b]) for b in range(B)]
    xk = [prep_x(key[b]) for b in range(B)]
    xv = [prep_x(value[b]) for b in range(B)]

    def prep_w(W, g):     # rows [256g, 256g+256) of W, transposed -> [128, 8, 256]
        sl = np.asarray(W, np.float32)[g * C:(g + 1) * C, :].T
        return _bf16(sl.reshape(DK, P, C).transpose(1, 0, 2))

    def prep_wo(g):       # Wo[:, 256g:256g+256].T -> [128, 2, 1024]
        sl = np.asarray(Wo, np.float32)[:, g * C:(g + 1) * C].T
        return _bf16(sl.reshape(C // P, P, D).transpose(1, 0, 2))

    def prep_b(b_, g):
        sl = np.asarray(b_, np.float32)[g * C:(g + 1) * C]
        return np.ascontiguousarray(sl.reshape(C // P, P).T)

    def prep_bvb(g):
        sl = np.asarray(bv, np.float32)[g * C:(g + 1) * C]
        return np.ascontiguousarray(np.broadcast_to(sl[None, :], (P, C)))

    strips = _strips()
    maskT = _bf16(m2d.T.astype(np.float32)) if mask_mode == "generic" else None

    in_maps = []
    for c in range(NCORES):
        b, g = c // GROUPS, c % GROUPS
        cb = np.concatenate([
            prep_w(Wq, g).reshape(P, 2048), prep_w(Wk, g).reshape(P, 2048),
            prep_w(Wv, g).reshape(P, 2048), prep_wo(g).reshape(P, 2048),
            strips.reshape(P, 2048)], axis=1)
        cf = np.concatenate([
            prep_b(bq, g), prep_b(bk, g), prep_bvb(g)], axis=1)
        im = dict(xq=xq[b], xk=xk[b], xv=xv[b],
                  cb=np.ascontiguousarray(cb),
                  cf=np.ascontiguousarray(cf.astype(np.float32)))
        if maskT is not None:
            im["maskT"] = maskT
        in_maps.append(im)

    return mask_mode, in_maps


def kernel(query, key, value, mask, Wq, bq, Wk, bk, Wv, bv, Wo, bo):
    mask_mode, in_maps = prepare(query, key, value, mask, Wq, bq, Wk, bk,
                                 Wv, bv, Wo, bo)
    nc = _get(mask_mode)
    res = run_bass_kernel_spmd(nc, in_maps, list(range(NCORES)))
    partials = np.stack([res.results[c]["o"] for c in range(NCORES)])
    out = partials.reshape(B, GROUPS, S, D).sum(axis=1)
    out = out + np.asarray(bo, np.float32)[None, None, :]
    return out.astype(np.float32)



# revision 3
# speedup vs baseline: 1.1034x; 1.1034x over previous
"""Multi-head attention (B=2, S=2048, D=1024, H=16, causal) on 8 Trainium2 cores.

Sharding: core c handles batch b = c // 4 and head group g = c % 4 (4 heads,
d_model column slice [256*g, 256*g+256)).  QKV projections are computed per
core against the full sequence of its batch; attention runs per head in a
"scores-transposed" [k, q] layout; the output projection produces a per-core
partial [S, D] that the host sums over the 4 head-group cores of each batch.

v3 perf structure:
- x is transposed on the HOST (numpy) to [DK, 128, S] so the device does only
  straight contiguous DMAs (v2 used on-device DMA-xbar transposes which gated
  the first 20us and stalled mid-kernel).
- Weights/biases load as separate sub-tile DMAs on SWDGE (gpsimd) so the
  first projection's weight chunk lands in ~1us.
- Scores matmuls for a head pair are explicitly row-tiled: heads 2h/2h+1 at
  PE tile positions (0,0)/(64,0) in 64x128 mode.
- P@V runs in fp8e4 DoubleRow perf mode: expT ([P, 2, SB]) and v
  ([P, 2, HC, P]) are pair-indexed along k-tiles, so each k-tile PAIR is one
  fused K=256-virtual matmul at 2x bf16 throughput.  exp writes fp8 directly
  (bias -3 keeps exp in fp8e4 range; softmax is shift-invariant).  The
  denominator comes from 64 ones-columns appended to V (rows 64:127 of the
  P@V output), already broadcast across partitions for the normalize.
- All PSUM->SBUF copies in the output projection go to DVE; ACT is reserved
  for exp (it is the secondary bottleneck).
"""

import functools
import numpy as np
import ml_dtypes

import concourse.bass as bass
import concourse.bacc as bacc
import concourse.tile as tile
import concourse.mybir as mybir
from concourse.bass_utils import run_bass_kernel_spmd

dt = mybir.dt
F32 = dt.float32
BF16 = dt.bfloat16
FP8 = dt.float8e4
AFT = mybir.ActivationFunctionType

B, S, D = 2, 2048, 1024
H, DH = 16, 64
NCORES = 8
GROUPS = NCORES // B            # 4 head-groups
HC = H // GROUPS                # 4 heads per core
C = HC * DH                     # 256 = per-core head-column slice
P = 128
DK = D // P                     # 8 d_in chunks
SB = 512                        # q-slab width
NSLAB = S // SB                 # 4
KT = S // P                     # 16 k tiles
SCALE = 1.0 / float(np.sqrt(DH))


def _build(mask_mode: str):
    """mask_mode: 'causal' | 'none' | 'generic'. Returns compiled Bacc."""
    assert mask_mode in ("causal", "none", "generic")
    nc = bacc.Bacc("TRN2", target_bir_lowering=False, debug=False)

    # host-transposed x: xT[o, p, s] = x[s, 128*o + p]
    xq_d = nc.dram_tensor("xq", [DK, P, S], BF16, kind="ExternalInput").ap()
    xk_d = nc.dram_tensor("xk", [DK, P, S], BF16, kind="ExternalInput").ap()
    xv_d = nc.dram_tensor("xv", [DK, P, S], BF16, kind="ExternalInput").ap()
    # all bf16 constants packed into one tensor: wq|wk|wv|wo|strips
    cb_d = nc.dram_tensor("cb", [P, 5 * 2048], BF16, kind="ExternalInput").ap()
    cf_d = nc.dram_tensor("cf", [P, 260], F32, kind="ExternalInput").ap()
    if mask_mode == "generic":
        maskT_d = nc.dram_tensor("maskT", [S, S], BF16, kind="ExternalInput").ap()
    o_d = nc.dram_tensor("o", [S, D], F32, kind="ExternalOutput").ap()

    with tile.TileContext(nc) as tc:
        with (
            tc.tile_pool(name="consts", bufs=1) as consts,
            tc.tile_pool(name="xT", bufs=3) as xT_pool,
            tc.tile_pool(name="acts", bufs=1) as acts,
            tc.tile_pool(name="expT", bufs=8) as exp_pool,
            tc.tile_pool(name="stage", bufs=2) as stage,
            tc.tile_pool(name="pp", bufs=2, space="PSUM") as pp,
            tc.tile_pool(name="sps", bufs=2, space="PSUM") as sps,
            tc.tile_pool(name="otp", bufs=2, space="PSUM") as otp,
        ):
            # ---- constants: split sub-tile DMAs on SWDGE, wq/wk/biases first
            cb_sb = consts.tile([P, 5 * 2048], BF16)
            cf_sb = consts.tile([P, 260], F32)
            nc.gpsimd.dma_start(cb_sb[:, 0:2048], cb_d[:, 0:2048])        # wq
            nc.gpsimd.dma_start(cb_sb[:, 2048:4096], cb_d[:, 2048:4096])  # wk
            nc.gpsimd.dma_start(cf_sb[:], cf_d)                           # biases
            nc.gpsimd.dma_start(cb_sb[:, 4096:6144], cb_d[:, 4096:6144])  # wv
            nc.gpsimd.dma_start(cb_sb[:, 8192:10240], cb_d[:, 8192:10240])  # strips
            nc.gpsimd.dma_start(cb_sb[:, 6144:8192], cb_d[:, 6144:8192])  # wo
            wq_sb = cb_sb[:, 0:2048].rearrange("p (o n) -> p o n", o=DK)
            wk_sb = cb_sb[:, 2048:4096].rearrange("p (o n) -> p o n", o=DK)
            wv_sb = cb_sb[:, 4096:6144].rearrange("p (o n) -> p o n", o=DK)
            wo_sb = cb_sb[:, 6144:8192].rearrange("p (c n) -> p c n", c=C // P)
            strips_sb = cb_sb[:, 8192:10240].rearrange("p (i n) -> p i n",
                                                       i=SB // P)
            bq_sb = cf_sb[:, 0:2]
            bk_sb = cf_sb[:, 2:4]
            bvb_sb = cf_sb[:, 4:260]
            expbias_sb = consts.tile([P, 1], F32)
            nc.vector.memset(expbias_sb[:], -3.0)

            # ---- x loads: straight DMAs, half-major so s-half-0 of q, k AND
            # v land first and slab-0 attention unblocks early.
            xqT = xT_pool.tile([P, DK, S], BF16, tag="xT", name="xqT")
            xkT = xT_pool.tile([P, DK, S], BF16, tag="xT", name="xkT")
            xvT = xT_pool.tile([P, DK, S], BF16, tag="xT", name="xvT")
            HS = S // 2
            for h in range(2):
                cs = slice(h * HS, (h + 1) * HS)
                for o in range(DK):
                    nc.sync.dma_start(xqT[:, o, cs], xq_d[o, :, cs])
                    nc.scalar.dma_start(xkT[:, o, cs], xk_d[o, :, cs])
                for o in range(DK):
                    eng = nc.sync if o % 2 == 0 else nc.scalar
                    eng.dma_start(xvT[:, o, cs], xv_d[o, :, cs])

            qT_sb = acts.tile([P, C // P, S], BF16)       # [d_out, s] head-major
            kT_sb = acts.tile([P, C // P, S], BF16)
            headsT_sb = acts.tile([P, C // P, S], BF16)
            # v pair-indexed fp8: [p, kpair, i, h, col]; cols 0:64 v, 64:128 ones
            v_sb = acts.tile([P, KT // 2, 2, HC, P], FP8)
            nc.vector.memset(v_sb[:, :, :, :, DH:P], 1.0)

            def proj_qk(jpair):
                # co-outer so attention head-pair 0 (co=0 of q AND k) unblocks
                # after half the work; j-paired so both matmuls of a j-pair
                # share the same stationary chunk.
                for co in range(C // P):
                    for (w_sb, b_sb, outT, xT) in ((wq_sb, bq_sb, qT_sb, xqT),
                                                   (wk_sb, bk_sb, kT_sb, xkT)):
                        ps = {}
                        for j in jpair:
                            ps[j] = pp.tile([P, SB], F32, tag="pp",
                                            name="proj_ps")
                        for o in range(DK):
                            for j in jpair:
                                nc.tensor.matmul(
                                    ps[j][:],
                                    lhsT=w_sb[:, o, co * P:(co + 1) * P],
                                    rhs=xT[:, o, j * SB:(j + 1) * SB],
                                    start=(o == 0), stop=(o == DK - 1))
                        for j in jpair:
                            nc.vector.tensor_scalar_add(
                                outT[:, co, j * SB:(j + 1) * SB], ps[j][:],
                                b_sb[:, co:co + 1])

            def vproj(st_range):
                for st in st_range:
                    ps = pp.tile([P, SB], F32, tag="pp", name="vproj_ps")
                    for o in range(DK):
                        nc.tensor.matmul(
                            ps[:, 0:C],
                            lhsT=xvT[:, o, st * P:(st + 1) * P],
                            rhs=wv_sb[:, o, :],
                            start=(o == 0), stop=(o == DK - 1))
                    nc.vector.tensor_add(
                        v_sb[:, st // 2, st % 2, :, 0:DH],
                        ps[:, 0:C].rearrange("p (h d) -> p h d", h=HC),
                        bvb_sb[:].rearrange("p (h d) -> p h d", h=HC))

            def attn_slab(j):
                n_kt = 4 * (j + 1) if mask_mode == "causal" else KT
                for hc in range(HC // 2):          # head pair (2hc, 2hc+1)
                    outp = [otp.tile([P, SB], F32, tag="otp", name=f"outp{hh}")
                            for hh in range(2)]
                    for tb in range(0, n_kt, 2):

                        def qlo(t):   # valid-q offset within slab for k-tile t
                            if mask_mode != "causal":
                                return 0
                            return max(0, P * t - SB * j)

                        sp = [sps.tile([P, 2, SB], F32, tag="sps",
                                       name=f"sp{hh}") for hh in range(2)]
                        expT = [exp_pool.tile([P, 2, SB], FP8, tag="expT",
                                              name=f"expT{hh}")
                                for hh in range(2)]
                        for d_ in range(2):
                            t = tb + d_
                            ql = qlo(t)
                            for hh in range(2):
                                hp = DH * hh
                                nc.tensor.matmul(
                                    sp[hh][:, d_, ql:],
                                    lhsT=kT_sb[hp:hp + DH, hc,
                                               t * P:(t + 1) * P],
                                    rhs=qT_sb[hp:hp + DH, hc,
                                              j * SB + ql:(j + 1) * SB],
                                    start=True, stop=True,
                                    tile_position=(hp, 0))
                        qb = qlo(tb)
                        for hh in range(2):
                            # bias -3: keeps exp within fp8e4 range (softmax
                            # is shift-invariant; numerator and denominator
                            # share the e^-3 factor)
                            nc.scalar.activation(
                                expT[hh][:, 0:2, qb:],
                                sp[hh][:, 0:2, qb:], AFT.Exp, scale=SCALE,
                                bias=expbias_sb[:])
                        qp = qlo(tb)      # pair-level valid-q offset
                        for d_ in range(2):
                            t = tb + d_
                            if mask_mode == "generic":
                                m_sb = stage.tile([P, SB], BF16, tag="msk",
                                                  name="m_sb")
                                nc.gpsimd.dma_start(
                                    m_sb[:],
                                    maskT_d[t * P:(t + 1) * P,
                                            j * SB:(j + 1) * SB])
                            for hh in range(2):
                                if mask_mode == "causal" and t >= 4 * j:
                                    # mask relative to the PAIR's q window:
                                    # d_=0: triangle on 128 cols; d_=1: zero
                                    # prefix + triangle over 256 cols
                                    w = min((d_ + 1) * P, SB - qp)
                                    nc.vector.tensor_mul(
                                        expT[hh][:, d_, qp:qp + w],
                                        expT[hh][:, d_, qp:qp + w],
                                        strips_sb[:, d_, 0:w])
                                elif mask_mode == "generic":
                                    nc.vector.tensor_mul(
                                        expT[hh][:, d_, :], expT[hh][:, d_, :],
                                        m_sb[:])
                        # fused P@V: one DoubleRow matmul per k-tile pair
                        # (2 fp8 weights per PE cell; virtual K=256)
                        for hh in range(2):
                            h = 2 * hc + hh
                            nc.tensor.matmul(
                                outp[hh][:, qp:],
                                lhsT=v_sb[:, tb // 2, :, h, :],
                                rhs=expT[hh][:, :, qp:],
                                start=(tb == 0), stop=(tb == n_kt - 2),
                                perf_mode=mybir.MatmulPerfMode.DoubleRow)
                    # normalize: rows 64:128 of outp hold the denominator.
                    # reciprocal_approx_fast mis-reads when in/out partition
                    # bases differ, so run it over all 128 rows (rows 0:64
                    # are recip of the unnormalized output — unused).
                    for hh in range(2):
                        hp = DH * hh
                        recip = stage.tile([P, SB], F32, tag="recip",
                                           name="recip")
                        nc.vector.reciprocal_approx_fast(recip[:], outp[hh][:])
                        nc.vector.tensor_mul(
                            headsT_sb[hp:hp + DH, hc, j * SB:(j + 1) * SB],
                            outp[hh][0:DH, :], recip[DH:P, :])

            def oproj_slab(j):
                for st in range(4 * j, 4 * j + 4):
                    ob = stage.tile([P, D], F32, tag="ob", name="ob")
                    ps = {}
                    for n2 in range(D // SB):
                        ps[n2] = pp.tile([P, SB], F32, tag="pp", name="o_ps")
                    # cc-outer: both n2 matmuls of a cc share the stationary
                    for cc in range(C // P):
                        for n2 in range(D // SB):
                            nc.tensor.matmul(
                                ps[n2][:],
                                lhsT=headsT_sb[:, cc, st * P:(st + 1) * P],
                                rhs=wo_sb[:, cc, n2 * SB:(n2 + 1) * SB],
                                start=(cc == 0), stop=(cc == C // P - 1))
                    for n2 in range(D // SB):
                        nc.vector.tensor_copy(ob[:, n2 * SB:(n2 + 1) * SB],
                                              ps[n2][:])
                    # stores go on the sync queue only: a store trigger waits
                    # for ob, and on the scalar queue that wait would block
                    # subsequent exp instructions (strict FIFO sequencer)
                    nc.sync.dma_start(o_d[st * P:(st + 1) * P, :], ob[:])

            # ---- interleaved schedule: projections feed attention per slab
            proj_qk((0, 1))
            vproj(range(0, 8))
            attn_slab(0)
            oproj_slab(0)
            proj_qk((2, 3))
            attn_slab(1)
            oproj_slab(1)
            vproj(range(8, KT))
            attn_slab(2)
            oproj_slab(2)
            attn_slab(3)
            oproj_slab(3)

    nc.compile()
    return nc


@functools.lru_cache(maxsize=4)
def _get(mask_mode: str):
    return _build(mask_mode)


def _bf16(a):
    return np.ascontiguousarray(a.astype(ml_dtypes.bfloat16))


def _detect_mask_mode(m):
    if (m == 1).all():
        return "none"
    idx = np.arange(m.shape[0])
    if np.array_equal(m != 0, idx[None, :] <= idx[:, None]):
        return "causal"
    return "generic"


def _strips():
    p = np.arange(P)[:, None]
    f = np.arange(SB)[None, :]
    s = np.stack([(p <= f - P * i) for i in range(SB // P)], axis=1)
    return np.ascontiguousarray(s.astype(ml_dtypes.bfloat16))


def prepare(query, key, value, mask, Wq, bq, Wk, bk, Wv, bv, Wo, bo):
    """Returns (mask_mode, in_maps) for run_bass_kernel_spmd."""
    query = np.asarray(query, dtype=np.float32)
    key = np.asarray(key, dtype=np.float32)
    value = np.asarray(value, dtype=np.float32)
    m2d = np.asarray(mask).reshape(np.asarray(mask).shape[-2:])
    mask_mode = _detect_mask_mode(m2d)

    def prep_x(x):    # [S, D] -> transposed [DK, P, S]
        return _bf16(np.ascontiguousarray(x.T).reshape(DK, P, S))

    xq = [prep_x(query[b]) for b in range(B)]
    xk = [prep_x(key[b]) for b in range(B)]
    xv = [prep_x(value[b]) for b in range(B)]

    def prep_w(W, g):     # rows [256g, 256g+256) of W, transposed -> [128, 8, 256]
        sl = np.asarray(W, np.float32)[g * C:(g + 1) * C, :].T
        return _bf16(sl.reshape(DK, P, C).transpose(1, 0, 2))

    def prep_wo(g):       # Wo[:, 256g:256g+256].T -> [128, 2, 1024]
        sl = np.asarray(Wo, np.float32)[:, g * C:(g + 1) * C].T
        return _bf16(sl.reshape(C // P, P, D).transpose(1, 0, 2))

    def prep_b(b_, g):
        sl = np.asarray(b_, np.float32)[g * C:(g + 1) * C]
        return np.ascontiguousarray(sl.reshape(C // P, P).T)

    def prep_bvb(g):
        sl = np.asarray(bv, np.float32)[g * C:(g + 1) * C]
        return np.ascontiguousarray(np.broadcast_to(sl[None, :], (P, C)))

    strips = _strips()
    maskT = _bf16(m2d.T.astype(np.float32)) if mask_mode == "generic" else None

    in_maps = []
    for c in range(NCORES):
        b, g = c // GROUPS, c % GROUPS
        cb = np.concatenate([
            prep_w(Wq, g).reshape(P, 2048), prep_w(Wk, g).reshape(P, 2048),
            prep_w(Wv, g).reshape(P, 2048), prep_wo(g).reshape(P, 2048),
            strips.reshape(P, 2048)], axis=1)
        cf = np.concatenate([
            prep_b(bq, g), prep_b(bk, g), prep_bvb(g)], axis=1)
        im = dict(xq=xq[b], xk=xk[b], xv=xv[b],
                  cb=np.ascontiguousarray(cb),
                  cf=np.ascontiguousarray(cf.astype(np.float32)))
        if maskT is not None:
            im["maskT"] = maskT
        in_maps.append(im)

    return mask_mode, in_maps


def kernel(query, key, value, mask, Wq, bq, Wk, bk, Wv, bv, Wo, bo):
    mask_mode, in_maps = prepare(query, key, value, mask, Wq, bq, Wk, bk,
                                 Wv, bv, Wo, bo)
    nc = _get(mask_mode)
    res = run_bass_kernel_spmd(nc, in_maps, list(range(NCORES)))
    partials = np.stack([res.results[c]["o"] for c in range(NCORES)])
    out = partials.reshape(B, GROUPS, S, D).sum(axis=1)
    out = out + np.asarray(bo, np.float32)[None, None, :]
    return out.astype(np.float32)
